# revision 1
# baseline (speedup 1.0000x reference)
"""Correlation-loss kernel for Trainium2 (8 NeuronCores, SPMD data-parallel).

Problem: for 800 random 16x16 patches of a 64-channel MSI image (first 32
channels used) and a 3-channel HE image, compute per-patch masked pairwise
squared-distance matrices over the 256 positions for both modalities and
L1-compare them; output sum(per-patch mean)/160.

Formulation: per patch, with mask m, sqd = sum_c msi^2 - sum_c he^2,
    out[a,b] = -(dm-dh)[a,b]/2 * m[a]m[b]
is a single rank-37 matmul lhsT.T @ rhs with
    lhsT = [xm*m (32) | -xh*m (3) | -sqd*m/2 | -m]   (K=37, cols=positions)
    rhs  = [xm*m (32) |  xh*m (3) |  m       | sqd*m/2]
and loss = sum_patches 2*sum|out| / 256^2 / 160. abs ignores the global
sign, so per-patch loss reduces to an abs-sum the chip does with ACT
(Abs+accumulator) and DVE (abs-reduce) straight out of PSUM.

Sharding: 100 patches per core, operands pre-gathered on host, partial
abs-sums returned per core, final scalar on host.
"""

import os
import sys

sys.path.insert(0, "/opt/trn_rl_repo")

import numpy as np

import concourse.bass as bass  # noqa: F401
import concourse.tile as tile
from concourse import bacc, mybir
from concourse.bass_utils import run_bass_kernel_spmd

WS = 16
NB = 800
TH = 0.05
P = WS * WS  # 256
N_CORES = 8
PPC = NB // N_CORES  # 100 patches per core
K = 37
BATCH = 4  # patches per PSUM group
NGROUP = PPC // BATCH  # 25

F32 = mybir.dt.float32
F32R = mybir.dt.float32r

LAST_EXEC_NS = None
LAST_RESULTS = None

_compiled = None


def _build_program():
    """Build + compile the per-core bass program (SPMD: same program, 8 cores)."""
    nc = bacc.Bacc(
        "TRN2", target_bir_lowering=False, debug=False, num_devices=N_CORES
    )

    lhs_d = nc.dram_tensor("lhs", [PPC, K, P], F32R, kind="ExternalInput").ap()
    rhs_d = nc.dram_tensor("rhs", [PPC, K, P], F32R, kind="ExternalInput").ap()
    out_d = nc.dram_tensor(
        "partial", [128, NGROUP + PPC], F32, kind="ExternalOutput"
    ).ap()

    with tile.TileContext(nc) as tc:
        with (
            tc.tile_pool(name="ops", bufs=3) as opool,
            tc.tile_pool(name="psum", bufs=2, space="PSUM") as ppool,
            tc.tile_pool(name="scratch", bufs=2) as spool,
            tc.tile_pool(name="accs", bufs=1) as apool,
        ):
            acc_a = apool.tile([128, NGROUP], F32)  # ACT accumulator slots
            acc_d = apool.tile([128, PPC], F32)  # DVE reduce slots

            for g in range(NGROUP):
                lhs_t = opool.tile([K, BATCH, P], F32R, tag="lhs")
                nc.sync.dma_start(
                    lhs_t[:],
                    lhs_d[g * BATCH : (g + 1) * BATCH].rearrange(
                        "b k c -> k b c"
                    ),
                )
                rhs_t = opool.tile([K, BATCH, P], F32R, tag="rhs")
                nc.sync.dma_start(
                    rhs_t[:],
                    rhs_d[g * BATCH : (g + 1) * BATCH].rearrange(
                        "b k c -> k b c"
                    ),
                )

                ps = ppool.tile([128, BATCH, 2 * P], F32)
                for p in range(BATCH):
                    nc.tensor.matmul(
                        ps[:, p, 0:P],
                        lhs_t[:, p, 0:128],
                        rhs_t[:, p, :],
                        start=True,
                        stop=True,
                    )
                    nc.tensor.matmul(
                        ps[:, p, P : 2 * P],
                        lhs_t[:, p, 128:256],
                        rhs_t[:, p, :],
                        start=True,
                        stop=True,
                    )

                # ACT: abs + accumulate over the chunk-0 halves [128, BATCH, 256]
                sc = spool.tile([128, BATCH * P], F32, tag="sc")
                nc.scalar.activation(
                    sc[:],
                    ps[:, :, 0:P],
                    mybir.ActivationFunctionType.Abs,
                    accum_out=acc_a[:, g : g + 1],
                )
                # DVE: abs-reduce over the chunk-1 halves
                nc.vector.tensor_reduce(
                    acc_d[:, g * BATCH : (g + 1) * BATCH],
                    ps[:, :, P : 2 * P],
                    axis=mybir.AxisListType.X,
                    op=mybir.AluOpType.add,
                    apply_absolute_value=True,
                )

            nc.sync.dma_start(out_d[:, 0:NGROUP], acc_a[:])
            nc.sync.dma_start(out_d[:, NGROUP:], acc_d[:])

    nc.compile()
    return nc


def _prep_operands(tensor_msi, tensor_he, i_idx, j_idx):
    """Host-side gather + operand construction. Returns lhsT/rhs [NB, K, P] f32."""
    msi = np.ascontiguousarray(tensor_msi[0, :32], dtype=np.float32)
    he = np.ascontiguousarray(tensor_he[0], dtype=np.float32)
    ii = np.asarray(i_idx).astype(np.int64)
    jj = np.asarray(j_idx).astype(np.int64)

    ig = np.broadcast_to(
        (ii[:, None] + np.arange(WS))[:, :, None], (NB, WS, WS)
    )
    jg = np.broadcast_to(
        (jj[:, None] + np.arange(WS))[:, None, :], (NB, WS, WS)
    )
    pm = msi[:, ig, jg].transpose(1, 0, 2, 3).reshape(NB, 32, P)
    ph = he[:, ig, jg].transpose(1, 0, 2, 3).reshape(NB, 3, P)

    m = (ph.sum(axis=1) >= TH).astype(np.float32)
    sq = (pm * pm).sum(1) - (ph * ph).sum(1)
    pm_m = pm * m[:, None]
    ph_m = ph * m[:, None]
    sqm = (0.5 * sq * m)[:, None]
    mm = m[:, None]

    lhsT = np.concatenate([pm_m, -ph_m, -sqm, -mm], axis=1).astype(np.float32)
    rhs = np.concatenate([pm_m, ph_m, mm, sqm], axis=1).astype(np.float32)
    return np.ascontiguousarray(lhsT), np.ascontiguousarray(rhs)


def kernel(tensor_msi, tensor_he, i_idx, j_idx, window_size, batch):
    global _compiled, LAST_EXEC_NS, LAST_RESULTS
    assert int(window_size) == WS and int(batch) == NB

    lhsT, rhs = _prep_operands(
        np.asarray(tensor_msi), np.asarray(tensor_he), i_idx, j_idx
    )

    if _compiled is None:
        _compiled = _build_program()
    nc = _compiled

    in_maps = [
        {
            "lhs": lhsT[c * PPC : (c + 1) * PPC],
            "rhs": rhs[c * PPC : (c + 1) * PPC],
        }
        for c in range(N_CORES)
    ]

    trace = bool(os.environ.get("KERNEL_TRACE"))
    res = run_bass_kernel_spmd(
        nc, in_maps, core_ids=list(range(N_CORES)), trace=trace
    )
    LAST_EXEC_NS = res.exec_time_ns
    LAST_RESULTS = res

    total = np.float64(0.0)
    for c in range(N_CORES):
        total += res.results[c]["partial"].astype(np.float64).sum()
    loss = total * 2.0 / (P * P) / (NB // 5)
    return np.float32(loss)


# revision 7
# speedup vs baseline: 1.3626x; 1.3626x over previous
"""Correlation-loss kernel for Trainium2 (8 NeuronCores, SPMD data-parallel).

Problem: for 800 random 16x16 patches of a 64-channel MSI image (first 32
channels used) and a 3-channel HE image, compute per-patch masked pairwise
squared-distance matrices over the 256 positions for both modalities and
L1-compare them; output sum(per-patch mean)/160.

Formulation: per patch, with mask m and sqd = sum_c msi^2 - sum_c he^2,
    out[a,b] = -(dm-dh)[a,b]/2 * m[a]m[b]
is a single rank-37 matmul lhsT.T @ rhs with
    lhsT = [xm*m (32) | -xh*m (3) | -sqd*m/2 | -m]   (K=37, cols=positions)
    rhs  = [xm*m (32) |  xh*m (3) |  m       | sqd*m/2]
and loss = sum_patches 2*sum|out| / 256^2 / 160 (abs kills the global sign).
out is symmetric, so only the upper 128-row chunk (D1|B, N=256) and the
lower-right diagonal block (D2, N=128) are computed. The double weight of
the off-diagonal block B is baked in on the host by doubling rhs columns
128:256 for the first matmul (a separate undoubled copy feeds the second),
so on-device everything is a single plain abs-sum taken straight out of
PSUM by ACT (Abs + accumulator) and DVE (abs-reduce) on disjoint patches
(= disjoint PSUM banks, keeping the engines parallel).

Sharding: 100 patches per core, operands pre-gathered and bf16-cast on host
in partition-major layout (fast 2D DMAs), partial sums returned per core.
"""

import os
import sys

sys.path.insert(0, "/opt/trn_rl_repo")

import ml_dtypes
import numpy as np

import concourse.bass as bass  # noqa: F401
import concourse.tile as tile
from concourse import bacc, mybir
from concourse.bass_utils import run_bass_kernel_spmd

WS = 16
NB = 800
TH = 0.05
P = WS * WS  # 256
HP = P // 2  # 128
N_CORES = 8
PPC = NB // N_CORES  # 100
K = 37
BATCH = 4  # patches per PSUM group (4 x 512 f32 = half of PSUM)
NGROUP = PPC // BATCH  # 25
NLOAD = 10  # input DMA chunks per tensor
ACT_PATCHES = 2  # patches per group summed by ACT (rest go to DVE)

F32 = mybir.dt.float32
BF16 = mybir.dt.bfloat16

LAST_EXEC_NS = None
LAST_RESULTS = None

_compiled = None


def _build_program():
    nc = bacc.Bacc(
        "TRN2", target_bir_lowering=False, debug=False, num_devices=N_CORES
    )

    lhs_d = nc.dram_tensor("lhs", [K, PPC, P], BF16, kind="ExternalInput").ap()
    rhsa_d = nc.dram_tensor("rhsa", [K, PPC, P], BF16, kind="ExternalInput").ap()
    rhsb_d = nc.dram_tensor("rhsb", [K, PPC, HP], BF16, kind="ExternalInput").ap()
    out_d = nc.dram_tensor("partial", [128, 1], F32, kind="ExternalOutput").ap()

    DVE_PATCHES = BATCH - ACT_PATCHES
    NSLOT = 1 + DVE_PATCHES  # per-group slot count (1 ACT + per-patch DVE)

    with tile.TileContext(nc) as tc:
        with (
            tc.tile_pool(name="ops", bufs=1) as opool,
            tc.tile_pool(name="psum", bufs=2, space="PSUM") as ppool,
            tc.tile_pool(name="accs", bufs=1) as apool,
            tc.tile_pool(name="scratch", bufs=2) as spool,
        ):
            lhs_all = opool.tile([K, PPC, P], BF16)
            rhsa_all = opool.tile([K, PPC, P], BF16)
            rhsb_all = opool.tile([K, PPC, HP], BF16)
            step = PPC // NLOAD
            for i in range(NLOAD):
                sl = slice(i * step, (i + 1) * step)
                nc.sync.dma_start(lhs_all[:, sl], lhs_d[:, sl])
                nc.sync.dma_start(rhsa_all[:, sl], rhsa_d[:, sl])
                nc.sync.dma_start(rhsb_all[:, sl], rhsb_d[:, sl])

            slots = apool.tile([128, NGROUP * NSLOT], F32)
            nc.vector.memset(slots[:], 0.0)

            for g in range(NGROUP):
                ps = ppool.tile([128, BATCH, 2 * P], F32)
                for pp in range(BATCH):
                    p = g * BATCH + pp
                    nc.tensor.matmul(
                        ps[:, pp, 0:P],
                        lhs_all[:, p, 0:HP],
                        rhsa_all[:, p, :],
                        start=True,
                        stop=True,
                    )
                    nc.tensor.matmul(
                        ps[:, pp, P : P + HP],
                        lhs_all[:, p, HP:P],
                        rhsb_all[:, p, :],
                        start=True,
                        stop=True,
                    )

                # per-patch psum cols 0:384 hold [D1 | 2B | D2]; plain abs-sum.
                # ACT takes the first ACT_PATCHES patches (their own PSUM
                # banks), DVE the rest -- disjoint banks, parallel engines.
                sc = spool.tile([128, ACT_PATCHES, 3 * HP], F32, tag="sc")
                nc.scalar.activation(
                    sc[:],
                    ps[:, 0:ACT_PATCHES, 0 : 3 * HP],
                    mybir.ActivationFunctionType.Abs,
                    accum_out=slots[:, g * NSLOT : g * NSLOT + 1],
                )
                nc.vector.tensor_reduce(
                    slots[:, g * NSLOT + 1 : (g + 1) * NSLOT],
                    ps[:, ACT_PATCHES:BATCH, 0 : 3 * HP],
                    axis=mybir.AxisListType.X,
                    op=mybir.AluOpType.add,
                    apply_absolute_value=True,
                )

            out_t = apool.tile([128, 1], F32)
            nc.vector.tensor_reduce(
                out_t[:],
                slots[:].rearrange("q (a c) -> q a c", a=1),
                axis=mybir.AxisListType.XY,
                op=mybir.AluOpType.add,
            )
            nc.sync.dma_start(out_d[:], out_t[:])

    nc.compile()
    return nc


def _prep_operands(tensor_msi, tensor_he, i_idx, j_idx):
    """Host gather + operand build.

    Returns lhs [N_CORES,K,PPC,P], rhsa (cols 128:256 doubled) same shape,
    rhsb [N_CORES,K,PPC,HP] (plain cols 128:256), all bf16 contiguous.
    """
    msi = np.ascontiguousarray(tensor_msi[0, :32], dtype=np.float32)
    he = np.ascontiguousarray(tensor_he[0], dtype=np.float32)
    ii = np.asarray(i_idx).astype(np.int64)
    jj = np.asarray(j_idx).astype(np.int64)

    ig = np.broadcast_to((ii[:, None] + np.arange(WS))[:, :, None], (NB, WS, WS))
    jg = np.broadcast_to((jj[:, None] + np.arange(WS))[:, None, :], (NB, WS, WS))
    pm = msi[:, ig, jg].transpose(1, 0, 2, 3).reshape(NB, 32, P)
    ph = he[:, ig, jg].transpose(1, 0, 2, 3).reshape(NB, 3, P)

    m = (ph.sum(axis=1) >= TH).astype(np.float32)
    sq = (pm * pm).sum(1) - (ph * ph).sum(1)
    pm_m = pm * m[:, None]
    ph_m = ph * m[:, None]
    sqm = (0.5 * sq * m)[:, None]
    mm = m[:, None]

    lhsT = np.concatenate([pm_m, -ph_m, -sqm, -mm], axis=1)  # [NB, K, P]
    rhs = np.concatenate([pm_m, ph_m, mm, sqm], axis=1)
    rhsa = rhs.copy()
    rhsa[:, :, HP:] *= 2.0
    rhsb = rhs[:, :, HP:]

    def shard(x):
        d = x.shape[2]
        y = x.reshape(N_CORES, PPC, K, d).transpose(0, 2, 1, 3)
        return np.ascontiguousarray(y.astype(ml_dtypes.bfloat16))

    return shard(lhsT), shard(rhsa), shard(rhsb)


def kernel(tensor_msi, tensor_he, i_idx, j_idx, window_size, batch):
    global _compiled, LAST_EXEC_NS, LAST_RESULTS
    assert int(window_size) == WS and int(batch) == NB

    lhs, rhsa, rhsb = _prep_operands(
        np.asarray(tensor_msi), np.asarray(tensor_he), i_idx, j_idx
    )

    if _compiled is None:
        _compiled = _build_program()
    nc = _compiled

    in_maps = [
        {"lhs": lhs[c], "rhsa": rhsa[c], "rhsb": rhsb[c]} for c in range(N_CORES)
    ]

    trace = bool(os.environ.get("KERNEL_TRACE"))
    res = run_bass_kernel_spmd(
        nc, in_maps, core_ids=list(range(N_CORES)), trace=trace
    )
    LAST_EXEC_NS = res.exec_time_ns
    LAST_RESULTS = res

    total = np.float64(0.0)
    for c in range(N_CORES):
        total += res.results[c]["partial"].astype(np.float64).sum()
    loss = total * 2.0 / (P * P) / (NB // 5)
    return np.float32(loss)


# revision 8
# speedup vs baseline: 4.0104x; 2.9432x over previous
"""Correlation-loss kernel for Trainium2 (8 NeuronCores, SPMD data-parallel).

Problem: for 800 random 16x16 patches of a 64-channel MSI image (first 32
channels used) and a 3-channel HE image, compute per-patch masked pairwise
squared-distance matrices over the 256 positions for both modalities and
L1-compare them; output sum(per-patch mean)/160.

Formulation: per patch, with mask m and sqd = sum_c msi^2 - sum_c he^2,
    out[a,b] = -(dm-dh)[a,b]/2 * m[a]m[b]
is a single rank-37 matmul lhsT.T @ rhs with
    lhsT = [xm*m (32) | -xh*m (3) | -sqd*m/2 | -m]   (K=37, cols=positions)
    rhs  = [xm*m (32) |  xh*m (3) |  m       | sqd*m/2]
and loss = sum_patches 2*sum|out| / 256^2 / 160 (abs kills the global sign).
out is symmetric, so only the upper 128-row chunk (D1|B, N=256) and the
lower-right diagonal block (D2, N=128) are computed. The double weight of
the off-diagonal block B is baked in on the host by doubling rhs columns
128:256 for the first matmul (a separate undoubled copy feeds the second),
so on-device everything is a single plain abs-sum taken straight out of
PSUM by ACT (Abs + accumulator) and DVE (abs-reduce) on disjoint patches
(= disjoint PSUM banks, keeping the engines parallel).

Memory layout: SBUF DMA bandwidth scales with the partition span of the
transfer (16 AXI ports x 8 partitions each), so 37-row operands are packed
two-per-128-partitions: even patches at partitions 0:37, odd at 64:101
(64 is the only legal matmul row offset for K=37), zeros between. DMAs
then run at full port width; odd-patch matmuls pass tile_position=(64,0).

Sharding: 100 patches per core, operands pre-gathered and bf16-cast on
host, partial sums returned per core, final scalar on host.
"""

import os
import sys

sys.path.insert(0, "/opt/trn_rl_repo")

import ml_dtypes
import numpy as np

import concourse.bass as bass  # noqa: F401
import concourse.tile as tile
from concourse import bacc, mybir
from concourse.bass_utils import run_bass_kernel_spmd

WS = 16
NB = 800
TH = 0.05
P = WS * WS  # 256
HP = P // 2  # 128
N_CORES = 8
PPC = NB // N_CORES  # 100
HPC = PPC // 2  # 50 patches per band
K = 37
BATCH = 4  # patches per PSUM group (4 x 512 f32 = half of PSUM)
NGROUP = PPC // BATCH  # 25
ACT_PATCHES = 2  # patches per group summed by ACT (rest go to DVE)
MEGA_CHUNKS = 5
RB_CHUNKS = 2

F32 = mybir.dt.float32
BF16 = mybir.dt.bfloat16

LAST_EXEC_NS = None
LAST_RESULTS = None

_compiled = None


def _build_program():
    nc = bacc.Bacc(
        "TRN2", target_bir_lowering=False, debug=False, num_devices=N_CORES
    )

    # mega: per half-patch h, cols [0:256)=lhs, [256:512)=rhsa(B cols doubled)
    # rows 0:37 even patches, 64:101 odd patches, zeros elsewhere
    mega_d = nc.dram_tensor("mega", [128, HPC, 2 * P], BF16, kind="ExternalInput").ap()
    rhsb_d = nc.dram_tensor("rhsb", [128, HPC, HP], BF16, kind="ExternalInput").ap()
    out_d = nc.dram_tensor("partial", [128, 1], F32, kind="ExternalOutput").ap()

    DVE_PATCHES = BATCH - ACT_PATCHES
    NSLOT = 1 + DVE_PATCHES

    with tile.TileContext(nc) as tc:
        with (
            tc.tile_pool(name="ops", bufs=1) as opool,
            tc.tile_pool(name="psum", bufs=2, space="PSUM") as ppool,
            tc.tile_pool(name="accs", bufs=1) as apool,
            tc.tile_pool(name="scratch", bufs=2) as spool,
        ):
            mega = opool.tile([128, HPC, 2 * P], BF16)
            rhsb = opool.tile([128, HPC, HP], BF16)
            step = HPC // MEGA_CHUNKS
            for i in range(MEGA_CHUNKS):
                sl = slice(i * step, (i + 1) * step)
                eng = nc.sync if i % 2 == 0 else nc.scalar
                eng.dma_start(mega[:, sl], mega_d[:, sl])
            rstep = HPC // RB_CHUNKS
            for i in range(RB_CHUNKS):
                sl = slice(i * rstep, (i + 1) * rstep)
                nc.scalar.dma_start(rhsb[:, sl], rhsb_d[:, sl])

            slots = apool.tile([128, NGROUP * NSLOT], F32)
            nc.vector.memset(slots[:], 0.0)

            for g in range(NGROUP):
                ps = ppool.tile([128, BATCH, 2 * P], F32)
                for pp in range(BATCH):
                    p = g * BATCH + pp
                    h = p // 2
                    if p % 2 == 0:
                        band = slice(0, K)
                        tp = None
                    else:
                        band = slice(64, 64 + K)
                        tp = (64, 0)
                    nc.tensor.matmul(
                        ps[:, pp, 0:P],
                        mega[band, h, 0:HP],
                        mega[band, h, P : 2 * P],
                        start=True,
                        stop=True,
                        tile_position=tp,
                    )
                    nc.tensor.matmul(
                        ps[:, pp, P : P + HP],
                        mega[band, h, HP:P],
                        rhsb[band, h, :],
                        start=True,
                        stop=True,
                        tile_position=tp,
                    )

                # per-patch psum cols 0:384 hold [D1 | 2B | D2]; plain abs-sum.
                sc = spool.tile([128, ACT_PATCHES, 3 * HP], F32, tag="sc")
                nc.scalar.activation(
                    sc[:],
                    ps[:, 0:ACT_PATCHES, 0 : 3 * HP],
                    mybir.ActivationFunctionType.Abs,
                    accum_out=slots[:, g * NSLOT : g * NSLOT + 1],
                )
                nc.vector.tensor_reduce(
                    slots[:, g * NSLOT + 1 : (g + 1) * NSLOT],
                    ps[:, ACT_PATCHES:BATCH, 0 : 3 * HP],
                    axis=mybir.AxisListType.X,
                    op=mybir.AluOpType.add,
                    apply_absolute_value=True,
                )

            out_t = apool.tile([128, 1], F32)
            nc.vector.tensor_reduce(
                out_t[:],
                slots[:].rearrange("q (a c) -> q a c", a=1),
                axis=mybir.AxisListType.XY,
                op=mybir.AluOpType.add,
            )
            nc.sync.dma_start(out_d[:], out_t[:])

    nc.compile()
    return nc


def _prep_operands(tensor_msi, tensor_he, i_idx, j_idx):
    """Host gather + operand build.

    Returns mega [N_CORES,128,HPC,2P] and rhsb [N_CORES,128,HPC,HP] bf16.
    """
    msi = np.ascontiguousarray(tensor_msi[0, :32], dtype=np.float32)
    he = np.ascontiguousarray(tensor_he[0], dtype=np.float32)
    ii = np.asarray(i_idx).astype(np.int64)
    jj = np.asarray(j_idx).astype(np.int64)

    ig = np.broadcast_to((ii[:, None] + np.arange(WS))[:, :, None], (NB, WS, WS))
    jg = np.broadcast_to((jj[:, None] + np.arange(WS))[:, None, :], (NB, WS, WS))
    pm = msi[:, ig, jg].transpose(1, 0, 2, 3).reshape(NB, 32, P)
    ph = he[:, ig, jg].transpose(1, 0, 2, 3).reshape(NB, 3, P)

    m = (ph.sum(axis=1) >= TH).astype(np.float32)
    sq = (pm * pm).sum(1) - (ph * ph).sum(1)
    pm_m = pm * m[:, None]
    ph_m = ph * m[:, None]
    sqm = (0.5 * sq * m)[:, None]
    mm = m[:, None]

    lhsT = np.concatenate([pm_m, -ph_m, -sqm, -mm], axis=1)  # [NB, K, P]
    rhs = np.concatenate([pm_m, ph_m, mm, sqm], axis=1)
    rhsa = rhs.copy()
    rhsa[:, :, HP:] *= 2.0
    rhsb = np.ascontiguousarray(rhs[:, :, HP:])

    lhsT = lhsT.reshape(N_CORES, PPC, K, P)
    rhsa = rhsa.reshape(N_CORES, PPC, K, P)
    rhsb = rhsb.reshape(N_CORES, PPC, K, HP)

    mega = np.zeros((N_CORES, 128, HPC, 2 * P), dtype=ml_dtypes.bfloat16)
    rb = np.zeros((N_CORES, 128, HPC, HP), dtype=ml_dtypes.bfloat16)
    for par, base in ((0, 0), (1, 64)):
        rows = slice(base, base + K)
        # [N_CORES, HPC, K, P] -> [N_CORES, K, HPC, P]
        mega[:, rows, :, 0:P] = (
            lhsT[:, par::2].transpose(0, 2, 1, 3).astype(ml_dtypes.bfloat16)
        )
        mega[:, rows, :, P : 2 * P] = (
            rhsa[:, par::2].transpose(0, 2, 1, 3).astype(ml_dtypes.bfloat16)
        )
        rb[:, rows] = (
            rhsb[:, par::2].transpose(0, 2, 1, 3).astype(ml_dtypes.bfloat16)
        )
    return np.ascontiguousarray(mega), np.ascontiguousarray(rb)


def kernel(tensor_msi, tensor_he, i_idx, j_idx, window_size, batch):
    global _compiled, LAST_EXEC_NS, LAST_RESULTS
    assert int(window_size) == WS and int(batch) == NB

    mega, rb = _prep_operands(
        np.asarray(tensor_msi), np.asarray(tensor_he), i_idx, j_idx
    )

    if _compiled is None:
        _compiled = _build_program()
    nc = _compiled

    in_maps = [{"mega": mega[c], "rhsb": rb[c]} for c in range(N_CORES)]

    trace = bool(os.environ.get("KERNEL_TRACE"))
    res = run_bass_kernel_spmd(
        nc, in_maps, core_ids=list(range(N_CORES)), trace=trace
    )
    LAST_EXEC_NS = res.exec_time_ns
    LAST_RESULTS = res

    total = np.float64(0.0)
    for c in range(N_CORES):
        total += res.results[c]["partial"].astype(np.float64).sum()
    loss = total * 2.0 / (P * P) / (NB // 5)
    return np.float32(loss)


# revision 12
# speedup vs baseline: 4.2201x; 1.0523x over previous
"""Correlation-loss kernel for Trainium2 (8 NeuronCores, SPMD data-parallel).

Problem: for 800 random 16x16 patches of a 64-channel MSI image (first 32
channels used) and a 3-channel HE image, compute per-patch masked pairwise
squared-distance matrices over the 256 positions for both modalities and
L1-compare them; output sum(per-patch mean)/160.

Formulation: per patch, with mask m and sqd = sum_c msi^2 - sum_c he^2,
    out[a,b] = -(dm-dh)[a,b]/2 * m[a]m[b]
is a single rank-37 matmul lhsT.T @ rhs with
    lhsT = [xm*m (32) | -xh*m (3) | -sqd*m/2 | -m]   (K=37, cols=positions)
    rhs  = [xm*m (32) |  xh*m (3) |  m       | sqd*m/2]
and loss = sum_patches 2*sum|out| / 256^2 / 160 (abs kills the global sign).
out is symmetric, so only the upper 128-row chunk (D1|B, N=256) and the
lower-right diagonal block (D2, N=128) are computed. The double weight of
the off-diagonal block B is baked in on the host by doubling rhs columns
128:256 for the first matmul (a separate undoubled copy feeds the second),
so on-device everything is a single plain abs-sum taken straight out of
PSUM by ACT (Abs + accumulator) and DVE (abs-reduce) on disjoint patches
(= disjoint PSUM banks, keeping the engines parallel).

Memory layout: SBUF DMA bandwidth scales with the partition span of the
transfer (16 AXI ports x 8 partitions each), so 37-row operands are packed
two-per-128-partitions: even patches at partitions 0:37, odd at 64:101
(64 is the only legal matmul row offset for K=37), zeros between. DMAs
then run at full port width; odd-patch matmuls pass tile_position=(64,0).

Sharding: 100 patches per core, operands pre-gathered and bf16-cast on
host, partial sums returned per core, final scalar on host.
"""

import os
import sys

sys.path.insert(0, "/opt/trn_rl_repo")

import ml_dtypes
import numpy as np

import concourse.bass as bass  # noqa: F401
import concourse.tile as tile
from concourse import bacc, mybir
from concourse.bass_utils import run_bass_kernel_spmd

WS = 16
NB = 800
TH = 0.05
P = WS * WS  # 256
HP = P // 2  # 128
N_CORES = 8
PPC = NB // N_CORES  # 100
HPC = PPC // 2  # 50 patches per band
K = 37
BATCH = 4  # patches per PSUM group (4 x 512 f32 = half of PSUM)
NGROUP = PPC // BATCH  # 25
# input DMA chunk ladder (in half-patches; small first chunks let compute
# start while the bulk is still in flight)
MEGA_LADDER = [2, 4, 8, 16, 20]
RB_LADDER = [4, 12, 34]

F32 = mybir.dt.float32
BF16 = mybir.dt.bfloat16

LAST_EXEC_NS = None
LAST_RESULTS = None

_compiled = None


def _build_program():
    nc = bacc.Bacc(
        "TRN2", target_bir_lowering=False, debug=False, num_devices=N_CORES
    )

    # mega: per half-patch h, cols [0:256)=lhs, [256:512)=rhsa(B cols doubled)
    # rows 0:37 even patches, 64:101 odd patches, zeros elsewhere
    mega_d = nc.dram_tensor("mega", [128, HPC, 2 * P], BF16, kind="ExternalInput").ap()
    rhsb_d = nc.dram_tensor("rhsb", [128, HPC, HP], BF16, kind="ExternalInput").ap()
    out_d = nc.dram_tensor("partial", [128, 1], F32, kind="ExternalOutput").ap()

    NSLOT = BATCH  # odd groups: one DVE slot per patch; even groups: 1 ACT slot

    with tile.TileContext(nc) as tc:
        with (
            tc.tile_pool(name="ops", bufs=1) as opool,
            tc.tile_pool(name="psum", bufs=2, space="PSUM") as ppool,
            tc.tile_pool(name="accs", bufs=1) as apool,
            tc.tile_pool(name="scratch", bufs=2) as spool,
        ):
            mega = opool.tile([128, HPC, 2 * P], BF16)
            rhsb = opool.tile([128, HPC, HP], BF16)
            off = 0
            for i, w in enumerate(MEGA_LADDER):
                sl = slice(off, off + w)
                off += w
                eng = nc.sync if i % 2 == 0 else nc.scalar
                eng.dma_start(mega[:, sl], mega_d[:, sl])
            off = 0
            for i, w in enumerate(RB_LADDER):
                sl = slice(off, off + w)
                off += w
                eng = nc.scalar if i % 2 == 0 else nc.sync
                eng.dma_start(rhsb[:, sl], rhsb_d[:, sl])

            slots = apool.tile([128, NGROUP * NSLOT], F32)
            nc.vector.memset(slots[:], 0.0)

            for g in range(NGROUP):
                ps = ppool.tile([128, BATCH, 2 * P], F32)
                for pp in range(BATCH):
                    p = g * BATCH + pp
                    h = p // 2
                    if p % 2 == 0:
                        band = slice(0, K)
                        tp = None
                    else:
                        band = slice(64, 64 + K)
                        tp = (64, 0)
                    nc.tensor.matmul(
                        ps[:, pp, 0:P],
                        mega[band, h, 0:HP],
                        mega[band, h, P : 2 * P],
                        start=True,
                        stop=True,
                        tile_position=tp,
                    )
                    nc.tensor.matmul(
                        ps[:, pp, P : P + HP],
                        mega[band, h, HP:P],
                        rhsb[band, h, :],
                        start=True,
                        stop=True,
                        tile_position=tp,
                    )

                # per-patch psum cols 0:384 hold [D1 | 2B | D2]; plain abs-sum.
                # Whole group goes to ONE engine (alternating): bigger ops,
                # half the semaphore traffic, banks never shared.
                if g % 2 == 0:
                    sc = spool.tile([128, BATCH, 3 * HP], F32, tag="sc")
                    nc.scalar.activation(
                        sc[:],
                        ps[:, :, 0 : 3 * HP],
                        mybir.ActivationFunctionType.Abs,
                        accum_out=slots[:, g * NSLOT : g * NSLOT + 1],
                    )
                else:
                    nc.vector.tensor_reduce(
                        slots[:, g * NSLOT : (g + 1) * NSLOT],
                        ps[:, :, 0 : 3 * HP],
                        axis=mybir.AxisListType.X,
                        op=mybir.AluOpType.add,
                        apply_absolute_value=True,
                    )

            out_t = apool.tile([128, 1], F32)
            nc.vector.tensor_reduce(
                out_t[:],
                slots[:].rearrange("q (a c) -> q a c", a=1),
                axis=mybir.AxisListType.XY,
                op=mybir.AluOpType.add,
            )
            nc.sync.dma_start(out_d[:], out_t[:])

    nc.compile()
    return nc


def _prep_operands(tensor_msi, tensor_he, i_idx, j_idx):
    """Host gather + operand build.

    Returns mega [N_CORES,128,HPC,2P] and rhsb [N_CORES,128,HPC,HP] bf16.
    """
    msi = np.ascontiguousarray(tensor_msi[0, :32], dtype=np.float32)
    he = np.ascontiguousarray(tensor_he[0], dtype=np.float32)
    ii = np.asarray(i_idx).astype(np.int64)
    jj = np.asarray(j_idx).astype(np.int64)

    ig = np.broadcast_to((ii[:, None] + np.arange(WS))[:, :, None], (NB, WS, WS))
    jg = np.broadcast_to((jj[:, None] + np.arange(WS))[:, None, :], (NB, WS, WS))
    pm = msi[:, ig, jg].transpose(1, 0, 2, 3).reshape(NB, 32, P)
    ph = he[:, ig, jg].transpose(1, 0, 2, 3).reshape(NB, 3, P)

    m = (ph.sum(axis=1) >= TH).astype(np.float32)
    sq = (pm * pm).sum(1) - (ph * ph).sum(1)
    pm_m = pm * m[:, None]
    ph_m = ph * m[:, None]
    sqm = (0.5 * sq * m)[:, None]
    mm = m[:, None]

    lhsT = np.concatenate([pm_m, -ph_m, -sqm, -mm], axis=1)  # [NB, K, P]
    rhs = np.concatenate([pm_m, ph_m, mm, sqm], axis=1)
    rhsa = rhs.copy()
    rhsa[:, :, HP:] *= 2.0
    rhsb = np.ascontiguousarray(rhs[:, :, HP:])

    lhsT = lhsT.reshape(N_CORES, PPC, K, P)
    rhsa = rhsa.reshape(N_CORES, PPC, K, P)
    rhsb = rhsb.reshape(N_CORES, PPC, K, HP)

    mega = np.zeros((N_CORES, 128, HPC, 2 * P), dtype=ml_dtypes.bfloat16)
    rb = np.zeros((N_CORES, 128, HPC, HP), dtype=ml_dtypes.bfloat16)
    for par, base in ((0, 0), (1, 64)):
        rows = slice(base, base + K)
        # [N_CORES, HPC, K, P] -> [N_CORES, K, HPC, P]
        mega[:, rows, :, 0:P] = (
            lhsT[:, par::2].transpose(0, 2, 1, 3).astype(ml_dtypes.bfloat16)
        )
        mega[:, rows, :, P : 2 * P] = (
            rhsa[:, par::2].transpose(0, 2, 1, 3).astype(ml_dtypes.bfloat16)
        )
        rb[:, rows] = (
            rhsb[:, par::2].transpose(0, 2, 1, 3).astype(ml_dtypes.bfloat16)
        )
    return np.ascontiguousarray(mega), np.ascontiguousarray(rb)


def kernel(tensor_msi, tensor_he, i_idx, j_idx, window_size, batch):
    global _compiled, LAST_EXEC_NS, LAST_RESULTS
    assert int(window_size) == WS and int(batch) == NB

    mega, rb = _prep_operands(
        np.asarray(tensor_msi), np.asarray(tensor_he), i_idx, j_idx
    )

    if _compiled is None:
        _compiled = _build_program()
    nc = _compiled

    in_maps = [{"mega": mega[c], "rhsb": rb[c]} for c in range(N_CORES)]

    trace = bool(os.environ.get("KERNEL_TRACE"))
    res = run_bass_kernel_spmd(
        nc, in_maps, core_ids=list(range(N_CORES)), trace=trace
    )
    LAST_EXEC_NS = res.exec_time_ns
    LAST_RESULTS = res

    total = np.float64(0.0)
    for c in range(N_CORES):
        total += res.results[c]["partial"].astype(np.float64).sum()
    loss = total * 2.0 / (P * P) / (NB // 5)
    return np.float32(loss)


# revision 16
# speedup vs baseline: 4.2366x; 1.0039x over previous
"""Correlation-loss kernel for Trainium2 (8 NeuronCores, SPMD data-parallel).

Problem: for 800 random 16x16 patches of a 64-channel MSI image (first 32
channels used) and a 3-channel HE image, compute per-patch masked pairwise
squared-distance matrices over the 256 positions for both modalities and
L1-compare them; output sum(per-patch mean)/160.

Formulation: per patch, with mask m and sqd = sum_c msi^2 - sum_c he^2,
    out[a,b] = -(dm-dh)[a,b]/2 * m[a]m[b]
is a single rank-37 matmul lhsT.T @ rhs with
    lhsT = [xm*m (32) | -xh*m (3) | -sqd*m/2 | -m]   (K=37, cols=positions)
    rhs  = [xm*m (32) |  xh*m (3) |  m       | sqd*m/2]
and loss = sum_patches 2*sum|out| / 256^2 / 160 (abs kills the global sign).
out is symmetric, so only the upper 128-row chunk (D1|B, N=256) and the
lower-right diagonal block (D2, N=128) are computed. The double weight of
the off-diagonal block B is baked in on the host by doubling rhs columns
128:256 for the first matmul (a separate undoubled copy feeds the second),
so on-device everything is a single plain abs-sum taken straight out of
PSUM by ACT (Abs + accumulator) and DVE (abs-reduce) on disjoint patches
(= disjoint PSUM banks, keeping the engines parallel).

Memory layout: SBUF DMA bandwidth scales with the partition span of the
transfer (16 AXI ports x 8 partitions each), so 37-row operands are packed
two-per-128-partitions: even patches at partitions 0:37, odd at 64:101
(64 is the only legal matmul row offset for K=37), zeros between. DMAs
then run at full port width; odd-patch matmuls pass tile_position=(64,0).

Sharding: 100 patches per core, operands pre-gathered and bf16-cast on
host, partial sums returned per core, final scalar on host.
"""

import os
import sys

sys.path.insert(0, "/opt/trn_rl_repo")

import ml_dtypes
import numpy as np

import concourse.bass as bass  # noqa: F401
import concourse.tile as tile
from concourse import bacc, mybir
from concourse.bass_utils import run_bass_kernel_spmd

WS = 16
NB = 800
TH = 0.05
P = WS * WS  # 256
HP = P // 2  # 128
N_CORES = 8
PPC = NB // N_CORES  # 100
HPC = PPC // 2  # 50 patches per band
K = 37
BATCH = 2  # patches per PSUM sub-group (2 x 512 f32 = 2 banks; bufs=4)
NGROUP = PPC // BATCH  # 50
# input DMA chunk ladder (in half-patches; small first chunks let compute
# start while the bulk is still in flight)
MEGA_LADDER = [1, 2, 3, 4, 8, 12, 20]
RB_LADDER = [2, 6, 16, 26]

F32 = mybir.dt.float32
BF16 = mybir.dt.bfloat16

LAST_EXEC_NS = None
LAST_RESULTS = None

_compiled = None


def _build_program():
    nc = bacc.Bacc(
        "TRN2", target_bir_lowering=False, debug=False, num_devices=N_CORES
    )

    # mega: per half-patch h, cols [0:256)=lhs, [256:512)=rhsa(B cols doubled)
    # rows 0:37 even patches, 64:101 odd patches, zeros elsewhere
    mega_d = nc.dram_tensor("mega", [128, HPC, 2 * P], BF16, kind="ExternalInput").ap()
    rhsb_d = nc.dram_tensor("rhsb", [128, HPC, HP], BF16, kind="ExternalInput").ap()
    out_d = nc.dram_tensor("partial", [128, 1], F32, kind="ExternalOutput").ap()

    NSLOT = BATCH  # DVE sub-groups: one slot per patch; ACT sub-groups: 1 slot

    with tile.TileContext(nc) as tc:
        with (
            tc.tile_pool(name="ops", bufs=1) as opool,
            tc.tile_pool(name="psum", bufs=4, space="PSUM") as ppool,
            tc.tile_pool(name="accs", bufs=1) as apool,
            tc.tile_pool(name="scratch", bufs=4) as spool,
        ):
            mega = opool.tile([128, HPC, 2 * P], BF16)
            rhsb = opool.tile([128, HPC, HP], BF16)
            # DMA only from engines that do no compute here: sync (HWDGE)
            # and gpsimd (SWDGE). A scalar-ring DMA would block ACTIVATEs
            # behind bulk transfers.
            off = 0
            for i, w in enumerate(MEGA_LADDER):
                sl = slice(off, off + w)
                off += w
                eng = nc.sync if i % 2 == 0 else nc.gpsimd
                eng.dma_start(mega[:, sl], mega_d[:, sl])
            off = 0
            for i, w in enumerate(RB_LADDER):
                sl = slice(off, off + w)
                off += w
                eng = nc.gpsimd if i % 2 == 0 else nc.sync
                eng.dma_start(rhsb[:, sl], rhsb_d[:, sl])

            slots = apool.tile([128, NGROUP * NSLOT], F32)
            nc.vector.memset(slots[:], 0.0)

            for g in range(NGROUP):
                ps = ppool.tile([128, BATCH, 2 * P], F32)
                for pp in range(BATCH):
                    p = g * BATCH + pp
                    h = p // 2
                    if p % 2 == 0:
                        band = slice(0, K)
                        tp = None
                    else:
                        band = slice(64, 64 + K)
                        tp = (64, 0)
                    nc.tensor.matmul(
                        ps[:, pp, 0:P],
                        mega[band, h, 0:HP],
                        mega[band, h, P : 2 * P],
                        start=True,
                        stop=True,
                        tile_position=tp,
                    )
                    nc.tensor.matmul(
                        ps[:, pp, P : P + HP],
                        mega[band, h, HP:P],
                        rhsb[band, h, :],
                        start=True,
                        stop=True,
                        tile_position=tp,
                    )

                # per-patch psum cols 0:384 hold [D1 | 2B | D2]; plain abs-sum.
                # Sub-group goes to ONE engine; alternate engines every TWO
                # sub-groups so each engine sees back-to-back ops while the
                # other covers the next pair. Banks never shared.
                if (g // 2) % 2 == 0:
                    sc = spool.tile([128, BATCH, 3 * HP], F32, tag="sc")
                    nc.scalar.activation(
                        sc[:],
                        ps[:, :, 0 : 3 * HP],
                        mybir.ActivationFunctionType.Abs,
                        accum_out=slots[:, g * NSLOT : g * NSLOT + 1],
                    )
                else:
                    nc.vector.tensor_reduce(
                        slots[:, g * NSLOT : (g + 1) * NSLOT],
                        ps[:, :, 0 : 3 * HP],
                        axis=mybir.AxisListType.X,
                        op=mybir.AluOpType.add,
                        apply_absolute_value=True,
                    )

            out_t = apool.tile([128, 1], F32)
            nc.vector.tensor_reduce(
                out_t[:],
                slots[:].rearrange("q (a c) -> q a c", a=1),
                axis=mybir.AxisListType.XY,
                op=mybir.AluOpType.add,
            )
            nc.sync.dma_start(out_d[:], out_t[:])

    nc.compile()
    return nc


def _prep_operands(tensor_msi, tensor_he, i_idx, j_idx):
    """Host gather + operand build.

    Returns mega [N_CORES,128,HPC,2P] and rhsb [N_CORES,128,HPC,HP] bf16.
    """
    msi = np.ascontiguousarray(tensor_msi[0, :32], dtype=np.float32)
    he = np.ascontiguousarray(tensor_he[0], dtype=np.float32)
    ii = np.asarray(i_idx).astype(np.int64)
    jj = np.asarray(j_idx).astype(np.int64)

    ig = np.broadcast_to((ii[:, None] + np.arange(WS))[:, :, None], (NB, WS, WS))
    jg = np.broadcast_to((jj[:, None] + np.arange(WS))[:, None, :], (NB, WS, WS))
    pm = msi[:, ig, jg].transpose(1, 0, 2, 3).reshape(NB, 32, P)
    ph = he[:, ig, jg].transpose(1, 0, 2, 3).reshape(NB, 3, P)

    m = (ph.sum(axis=1) >= TH).astype(np.float32)
    sq = (pm * pm).sum(1) - (ph * ph).sum(1)
    pm_m = pm * m[:, None]
    ph_m = ph * m[:, None]
    sqm = (0.5 * sq * m)[:, None]
    mm = m[:, None]

    lhsT = np.concatenate([pm_m, -ph_m, -sqm, -mm], axis=1)  # [NB, K, P]
    rhs = np.concatenate([pm_m, ph_m, mm, sqm], axis=1)
    rhsa = rhs.copy()
    rhsa[:, :, HP:] *= 2.0
    rhsb = np.ascontiguousarray(rhs[:, :, HP:])

    lhsT = lhsT.reshape(N_CORES, PPC, K, P)
    rhsa = rhsa.reshape(N_CORES, PPC, K, P)
    rhsb = rhsb.reshape(N_CORES, PPC, K, HP)

    mega = np.zeros((N_CORES, 128, HPC, 2 * P), dtype=ml_dtypes.bfloat16)
    rb = np.zeros((N_CORES, 128, HPC, HP), dtype=ml_dtypes.bfloat16)
    for par, base in ((0, 0), (1, 64)):
        rows = slice(base, base + K)
        # [N_CORES, HPC, K, P] -> [N_CORES, K, HPC, P]
        mega[:, rows, :, 0:P] = (
            lhsT[:, par::2].transpose(0, 2, 1, 3).astype(ml_dtypes.bfloat16)
        )
        mega[:, rows, :, P : 2 * P] = (
            rhsa[:, par::2].transpose(0, 2, 1, 3).astype(ml_dtypes.bfloat16)
        )
        rb[:, rows] = (
            rhsb[:, par::2].transpose(0, 2, 1, 3).astype(ml_dtypes.bfloat16)
        )
    return np.ascontiguousarray(mega), np.ascontiguousarray(rb)


def kernel(tensor_msi, tensor_he, i_idx, j_idx, window_size, batch):
    global _compiled, LAST_EXEC_NS, LAST_RESULTS
    assert int(window_size) == WS and int(batch) == NB

    mega, rb = _prep_operands(
        np.asarray(tensor_msi), np.asarray(tensor_he), i_idx, j_idx
    )

    if _compiled is None:
        _compiled = _build_program()
    nc = _compiled

    in_maps = [{"mega": mega[c], "rhsb": rb[c]} for c in range(N_CORES)]

    trace = bool(os.environ.get("KERNEL_TRACE"))
    res = run_bass_kernel_spmd(
        nc, in_maps, core_ids=list(range(N_CORES)), trace=trace
    )
    LAST_EXEC_NS = res.exec_time_ns
    LAST_RESULTS = res

    total = np.float64(0.0)
    for c in range(N_CORES):
        total += res.results[c]["partial"].astype(np.float64).sum()
    loss = total * 2.0 / (P * P) / (NB // 5)
    return np.float32(loss)


# revision 18
# speedup vs baseline: 5.0376x; 1.1890x over previous
"""Correlation-loss kernel for Trainium2 (8 NeuronCores, SPMD data-parallel).

Problem: for 800 random 16x16 patches of a 64-channel MSI image (first 32
channels used) and a 3-channel HE image, compute per-patch masked pairwise
squared-distance matrices over the 256 positions for both modalities and
L1-compare them; output sum(per-patch mean)/160.

Formulation: per patch, with mask m and sqd = sum_c msi^2 - sum_c he^2,
    out[a,b] = -(dm-dh)[a,b]/2 * m[a]m[b]
is a single rank-37 matmul lhsT.T @ rhs with
    lhsT = [xm*m (32) | -xh*m (3) | -sqd*m/2 | -m]   (K=37, cols=positions)
    rhs  = [xm*m (32) |  xh*m (3) |  m       | sqd*m/2]
and loss = sum_patches 2*sum|out| / 256^2 / 160 (abs kills the global sign).
out is symmetric, so only the upper 128-row chunk (D1|B, N=256) and the
lower-right diagonal block (D2, N=128) are computed. The double weight of
the off-diagonal block B is baked in on the host by doubling rhs columns
128:256 for the first matmul (a separate undoubled copy feeds the second),
so on-device everything is a single plain abs-sum taken straight out of
PSUM by ACT (Abs + accumulator) and DVE (abs-reduce) on disjoint patches
(= disjoint PSUM banks, keeping the engines parallel).

Memory layout: SBUF DMA bandwidth scales with the partition span of the
transfer (16 AXI ports x 8 partitions each), so 37-row operands are packed
two-per-128-partitions: even patches at partitions 0:37, odd at 64:101
(64 is the only legal matmul row offset for K=37), zeros between. DMAs
then run at full port width; odd-patch matmuls pass tile_position=(64,0).

Sharding: 100 patches per core, operands pre-gathered and bf16-cast on
host, partial sums returned per core, final scalar on host.
"""

import os
import sys

sys.path.insert(0, "/opt/trn_rl_repo")

import ml_dtypes
import numpy as np

import concourse.bass as bass  # noqa: F401
import concourse.tile as tile
from concourse import bacc, mybir
from concourse.bass_utils import run_bass_kernel_spmd

WS = 16
NB = 800
TH = 0.05
P = WS * WS  # 256
HP = P // 2  # 128
N_CORES = 8
PPC = NB // N_CORES  # 100
HPC = PPC // 2  # 50 patches per band
K = 37
BATCH = 2  # patches per PSUM sub-group (2 x 512 f32 = 2 banks; bufs=4)
NGROUP = PPC // BATCH  # 50
# input DMA chunk ladder (in half-patches; small first chunks let compute
# start while the bulk is still in flight). All chunks go down the single
# sync HWDGE ring in consumption order (mega[k] then rhsb[k] per segment).
DMA_LADDER = [1, 1, 2, 4, 8, 14, 20]

F32 = mybir.dt.float32
BF16 = mybir.dt.bfloat16

LAST_EXEC_NS = None
LAST_RESULTS = None

_compiled = None


def _build_program():
    nc = bacc.Bacc(
        "TRN2", target_bir_lowering=False, debug=False, num_devices=N_CORES
    )

    # mega: per half-patch h, cols [0:256)=lhs, [256:512)=rhsa(B cols doubled)
    # rows 0:37 even patches, 64:101 odd patches, zeros elsewhere
    mega_d = nc.dram_tensor("mega", [128, HPC, 2 * P], BF16, kind="ExternalInput").ap()
    rhsb_d = nc.dram_tensor("rhsb", [128, HPC, HP], BF16, kind="ExternalInput").ap()
    out_d = nc.dram_tensor("partial", [128, 1], F32, kind="ExternalOutput").ap()

    NSLOT = BATCH  # DVE sub-groups: one slot per patch; ACT sub-groups: 1 slot

    with tile.TileContext(nc) as tc:
        with (
            tc.tile_pool(name="ops", bufs=1) as opool,
            tc.tile_pool(name="psum", bufs=4, space="PSUM") as ppool,
            tc.tile_pool(name="accs", bufs=1) as apool,
            tc.tile_pool(name="scratch", bufs=4) as spool,
        ):
            mega = opool.tile([128, HPC, 2 * P], BF16)
            rhsb = opool.tile([128, HPC, HP], BF16)
            # All input DMAs on the sync HWDGE ring, interleaved in
            # consumption order. (scalar ring would block ACTIVATEs behind
            # bulk transfers; gpsimd SWDGE transfers are far slower.)
            off = 0
            for w in DMA_LADDER:
                sl = slice(off, off + w)
                off += w
                nc.sync.dma_start(mega[:, sl], mega_d[:, sl])
                nc.sync.dma_start(rhsb[:, sl], rhsb_d[:, sl])

            slots = apool.tile([128, NGROUP * NSLOT], F32)
            nc.vector.memset(slots[:], 0.0)

            for g in range(NGROUP):
                ps = ppool.tile([128, BATCH, 2 * P], F32)
                for pp in range(BATCH):
                    p = g * BATCH + pp
                    h = p // 2
                    if p % 2 == 0:
                        band = slice(0, K)
                        tp = None
                    else:
                        band = slice(64, 64 + K)
                        tp = (64, 0)
                    nc.tensor.matmul(
                        ps[:, pp, 0:P],
                        mega[band, h, 0:HP],
                        mega[band, h, P : 2 * P],
                        start=True,
                        stop=True,
                        tile_position=tp,
                    )
                    nc.tensor.matmul(
                        ps[:, pp, P : P + HP],
                        mega[band, h, HP:P],
                        rhsb[band, h, :],
                        start=True,
                        stop=True,
                        tile_position=tp,
                    )

                # per-patch psum cols 0:384 hold [D1 | 2B | D2]; plain abs-sum.
                # Sub-group goes to ONE engine; alternate engines every TWO
                # sub-groups so each engine sees back-to-back ops while the
                # other covers the next pair. Banks never shared.
                if (g // 2) % 2 == 0:
                    sc = spool.tile([128, BATCH, 3 * HP], F32, tag="sc")
                    nc.scalar.activation(
                        sc[:],
                        ps[:, :, 0 : 3 * HP],
                        mybir.ActivationFunctionType.Abs,
                        accum_out=slots[:, g * NSLOT : g * NSLOT + 1],
                    )
                else:
                    nc.vector.tensor_reduce(
                        slots[:, g * NSLOT : (g + 1) * NSLOT],
                        ps[:, :, 0 : 3 * HP],
                        axis=mybir.AxisListType.X,
                        op=mybir.AluOpType.add,
                        apply_absolute_value=True,
                    )

            out_t = apool.tile([128, 1], F32)
            nc.vector.tensor_reduce(
                out_t[:],
                slots[:].rearrange("q (a c) -> q a c", a=1),
                axis=mybir.AxisListType.XY,
                op=mybir.AluOpType.add,
            )
            nc.sync.dma_start(out_d[:], out_t[:])

    nc.compile()
    return nc


def _prep_operands(tensor_msi, tensor_he, i_idx, j_idx):
    """Host gather + operand build.

    Returns mega [N_CORES,128,HPC,2P] and rhsb [N_CORES,128,HPC,HP] bf16.
    """
    msi = np.ascontiguousarray(tensor_msi[0, :32], dtype=np.float32)
    he = np.ascontiguousarray(tensor_he[0], dtype=np.float32)
    ii = np.asarray(i_idx).astype(np.int64)
    jj = np.asarray(j_idx).astype(np.int64)

    ig = np.broadcast_to((ii[:, None] + np.arange(WS))[:, :, None], (NB, WS, WS))
    jg = np.broadcast_to((jj[:, None] + np.arange(WS))[:, None, :], (NB, WS, WS))
    pm = msi[:, ig, jg].transpose(1, 0, 2, 3).reshape(NB, 32, P)
    ph = he[:, ig, jg].transpose(1, 0, 2, 3).reshape(NB, 3, P)

    m = (ph.sum(axis=1) >= TH).astype(np.float32)
    sq = (pm * pm).sum(1) - (ph * ph).sum(1)
    pm_m = pm * m[:, None]
    ph_m = ph * m[:, None]
    sqm = (0.5 * sq * m)[:, None]
    mm = m[:, None]

    lhsT = np.concatenate([pm_m, -ph_m, -sqm, -mm], axis=1)  # [NB, K, P]
    rhs = np.concatenate([pm_m, ph_m, mm, sqm], axis=1)
    rhsa = rhs.copy()
    rhsa[:, :, HP:] *= 2.0
    rhsb = np.ascontiguousarray(rhs[:, :, HP:])

    lhsT = lhsT.reshape(N_CORES, PPC, K, P)
    rhsa = rhsa.reshape(N_CORES, PPC, K, P)
    rhsb = rhsb.reshape(N_CORES, PPC, K, HP)

    mega = np.zeros((N_CORES, 128, HPC, 2 * P), dtype=ml_dtypes.bfloat16)
    rb = np.zeros((N_CORES, 128, HPC, HP), dtype=ml_dtypes.bfloat16)
    for par, base in ((0, 0), (1, 64)):
        rows = slice(base, base + K)
        # [N_CORES, HPC, K, P] -> [N_CORES, K, HPC, P]
        mega[:, rows, :, 0:P] = (
            lhsT[:, par::2].transpose(0, 2, 1, 3).astype(ml_dtypes.bfloat16)
        )
        mega[:, rows, :, P : 2 * P] = (
            rhsa[:, par::2].transpose(0, 2, 1, 3).astype(ml_dtypes.bfloat16)
        )
        rb[:, rows] = (
            rhsb[:, par::2].transpose(0, 2, 1, 3).astype(ml_dtypes.bfloat16)
        )
    return np.ascontiguousarray(mega), np.ascontiguousarray(rb)


def kernel(tensor_msi, tensor_he, i_idx, j_idx, window_size, batch):
    global _compiled, LAST_EXEC_NS, LAST_RESULTS
    assert int(window_size) == WS and int(batch) == NB

    mega, rb = _prep_operands(
        np.asarray(tensor_msi), np.asarray(tensor_he), i_idx, j_idx
    )

    if _compiled is None:
        _compiled = _build_program()
    nc = _compiled

    in_maps = [{"mega": mega[c], "rhsb": rb[c]} for c in range(N_CORES)]

    trace = bool(os.environ.get("KERNEL_TRACE"))
    res = run_bass_kernel_spmd(
        nc, in_maps, core_ids=list(range(N_CORES)), trace=trace
    )
    LAST_EXEC_NS = res.exec_time_ns
    LAST_RESULTS = res

    total = np.float64(0.0)
    for c in range(N_CORES):
        total += res.results[c]["partial"].astype(np.float64).sum()
    loss = total * 2.0 / (P * P) / (NB // 5)
    return np.float32(loss)
